# revision 22
# baseline (speedup 1.0000x reference)
"""MathildeGCN Trainium2 kernel: 7-layer GCN + global mean pool + linear head.

Strategy (8 NeuronCores, SPMD):
  - Nodes sharded contiguously: 12500/core, padded to 12544 = 98 tiles of 128.
  - Edges partitioned by dst shard, sorted by dst, bucketed into 32-node
    windows so the segment-sum becomes static-offset PSUM matmuls with
    one-hot (norm-scaled) selection matrices built on the vector engine.
  - Per layer: local H @ W (via PE transposes), AllGather of H' into a full
    gather table in DRAM, indirect-DMA gather of edge source rows, selection
    matmuls accumulate messages per dst tile, bias via rank-1 matmul,
    LayerNorm + ReLU + residual on-chip.
  - Mean-pool partials per graph via one-hot matmuls, AllReduce, final dot.
"""

import sys

sys.path.insert(0, "/opt/trn_rl_repo")

import numpy as np

import concourse.bass as bass
import concourse.bacc as bacc
import concourse.mybir as mybir
import concourse.tile as tile
from concourse.bass_utils import run_bass_kernel_spmd
from concourse.masks import make_identity

F32 = mybir.dt.float32
I32 = mybir.dt.int32

N_NODES = 100000
N_EDGES = 1600000
F = 128
N_GRAPHS = 1000
EPS = 1e-5
NC = 8
LOCAL = N_NODES // NC          # 12500
NT = (LOCAL + 127) // 128      # 98 tiles per core
LPAD = NT * 128                # 12544
GPAD = LPAD * NC               # 100352
WIN = 64                       # dst window width (selection matrix cols)
NWIN = 128 // WIN              # windows per tile (4)
NBUK = NT * NWIN               # buckets per core (392)
N_LAYERS = 7
GH = 512                       # graph window for pooling matmuls
NH = (N_GRAPHS + GH - 1) // GH


def _preprocess(edge_index, batch):
    """Build per-core edge arrays. Returns (per_core dict list, CPW)."""
    src = np.concatenate([edge_index[0], np.arange(N_NODES, dtype=np.int64)])
    dst = np.concatenate([edge_index[1], np.arange(N_NODES, dtype=np.int64)])
    deg = np.bincount(dst, minlength=N_NODES).astype(np.float64)
    dinv = np.where(deg > 0, 1.0 / np.sqrt(deg), 0.0)
    norm = (dinv[src] * dinv[dst]).astype(np.float32)
    # padded global ids for the gather table
    src_gid = ((src // LOCAL) * LPAD + (src % LOCAL)).astype(np.int32)

    cores = []
    all_counts = []
    for c in range(NC):
        lo, hi = c * LOCAL, (c + 1) * LOCAL
        m = (dst >= lo) & (dst < hi)
        ed = (dst[m] - lo).astype(np.int64)
        es = src_gid[m]
        en = norm[m]
        order = np.argsort(ed, kind="stable")
        ed, es, en = ed[order], es[order], en[order]
        buk = ed // WIN                      # bucket id 0..NBUK-1, sorted
        rel = (ed % WIN).astype(np.float32)  # dst index within window
        starts = np.searchsorted(buk, np.arange(NBUK))
        ends = np.searchsorted(buk, np.arange(NBUK) + 1)
        counts = ends - starts
        all_counts.append(counts)
        cores.append(dict(es=es, en=en, rel=rel, starts=starts, counts=counts))

    maxcnt = max(int(cnt.max()) for cnt in all_counts)
    CPW = (maxcnt + 127) // 128  # chunks per window (uniform, SPMD-safe)

    per_core = []
    for c in range(NC):
        d = cores[c]
        cap = CPW * 128
        esP = np.zeros((NBUK, cap), dtype=np.int32)
        enP = np.zeros((NBUK, cap), dtype=np.float32)
        relP = np.zeros((NBUK, cap), dtype=np.float32)
        pos = np.arange(len(d["es"])) - np.repeat(d["starts"], d["counts"])
        bix = np.repeat(np.arange(NBUK), d["counts"])
        esP[bix, pos] = d["es"]
        enP[bix, pos] = d["en"]
        relP[bix, pos] = d["rel"]

        # SBUF layout [128 lanes, NBUK*CPW cols]: col j = bucket*CPW + chunk
        def lanes(a):
            return np.ascontiguousarray(a.reshape(NBUK * CPW, 128).T)

        per_core.append(dict(esrc=lanes(esP), enorm=lanes(enP), erel=lanes(relP)))
    return per_core, CPW


def _pool_plan(batch):
    """Union over cores of per-tile graph halves (width GH) the tile touches."""
    plan = [set() for _ in range(NT)]
    for c in range(NC):
        lo = c * LOCAL
        for t in range(NT):
            a = lo + t * 128
            b = min(a + 128, lo + LOCAL)
            if a >= lo + LOCAL:
                continue
            gmin = int(batch[a]) // GH
            gmax = int(batch[b - 1]) // GH
            for h in range(gmin, min(gmax, NH - 1) + 1):
                plan[t].add(h)
    return [sorted(s) for s in plan]


def _bcast_inner(ap, reps):
    """Append a 0-stride inner dim of size `reps` to an AP."""
    new_ap = [list(p) for p in ap.ap] + [[0, reps]]
    return bass.AP(ap.tensor, ap.offset, new_ap)


def _build_nc(CPW, pool_plan, debug=False):
    NJ = NBUK * CPW          # edge-metadata columns in SBUF
    TJ = NWIN * CPW          # chunks per tile
    nc = bacc.Bacc(None, target_bir_lowering=False, num_devices=NC)

    # ---- I/O ----
    x_sh = nc.dram_tensor("x_sh", [LPAD, F], F32, kind="ExternalInput")
    esrc_d = nc.dram_tensor("esrc", [128, NJ], I32, kind="ExternalInput")
    enorm_d = nc.dram_tensor("enorm", [128, NJ], F32, kind="ExternalInput")
    erel_d = nc.dram_tensor("erel", [128, NJ], F32, kind="ExternalInput")
    batch_d = nc.dram_tensor("batchv", [128, NT], F32, kind="ExternalInput")
    wall_d = nc.dram_tensor("wall", [F, N_LAYERS * F], F32, kind="ExternalInput")
    gamma_d = nc.dram_tensor("gammar", [128, N_LAYERS * F], F32, kind="ExternalInput")
    beta_d = nc.dram_tensor("betar", [128, N_LAYERS * F], F32, kind="ExternalInput")
    bs_d = nc.dram_tensor("bsr", [1, N_LAYERS * F], F32, kind="ExternalInput")
    linw_d = nc.dram_tensor("linw", [F, 1], F32, kind="ExternalInput")
    linb_d = nc.dram_tensor("linb", [1, 1], F32, kind="ExternalInput")
    out_d = nc.dram_tensor("out", [N_GRAPHS, 1], F32, kind="ExternalOutput")
    if debug:
        dbg_hp = nc.dram_tensor("dbg_hp", [LPAD, F], F32, kind="ExternalOutput")
        dbg_h1 = nc.dram_tensor("dbg_h1", [LPAD, F], F32, kind="ExternalOutput")

    # per-layer AllGather buffers (dedicated tensors: offset-0 APs for the
    # indirect gather, and no cross-layer WAR hazards)
    hp_local = [
        nc.dram_tensor(f"hploc{l}", [LPAD, F], F32) for l in range(N_LAYERS)
    ]
    hp_full = [
        nc.dram_tensor(f"hpfull{l}", [GPAD, F], F32, addr_space="Shared")
        for l in range(N_LAYERS)
    ]
    ar_in = nc.dram_tensor("ar_in", [129, NH * GH], F32)
    ar_out = nc.dram_tensor("ar_out", [129, NH * GH], F32, addr_space="Shared")

    with tile.TileContext(nc) as tc:
        with tc.tile_pool(name="const", bufs=1) as constp:
            # ---- persistent SBUF ----
            H = constp.tile([128, NT * F], F32, tag="H")
            esrc_s = constp.tile([128, NJ], I32, tag="esrc")
            enorm_s = constp.tile([128, NJ], F32, tag="enorm")
            erel_s = constp.tile([128, NJ], F32, tag="erel")
            batch_s = constp.tile([128, NT], F32, tag="batch")
            wall_s = constp.tile([F, N_LAYERS * F], F32, tag="wall")
            gamma_s = constp.tile([128, N_LAYERS * F], F32, tag="gamma")
            beta_s = constp.tile([128, N_LAYERS * F], F32, tag="beta")
            bs_s = constp.tile([1, N_LAYERS * F], F32, tag="bs")
            linw_s = constp.tile([F, 1], F32, tag="linw")
            linb_s = constp.tile([1, 1], F32, tag="linb")
            ident = constp.tile([128, 128], F32, tag="ident")
            ones1 = constp.tile([1, 128], F32, tag="ones1")
            ones128 = constp.tile([128, 1], F32, tag="ones128")
            iota_w = constp.tile([128, TJ * WIN], F32, tag="iotaw")
            iota_g = constp.tile([128, NH * GH], F32, tag="iotag")
            eps_s = constp.tile([128, 1], F32, tag="eps")
            if globals().get("GATHER_MODE") == "skip":
                gatfix = constp.tile([128, NWIN * CPW * 128], F32, tag="gatfix")
                nc.gpsimd.memset(gatfix[:], 1.0)

            nc.sync.dma_start(esrc_s[:], esrc_d[:])
            nc.sync.dma_start(enorm_s[:], enorm_d[:])
            nc.sync.dma_start(erel_s[:], erel_d[:])
            nc.sync.dma_start(batch_s[:], batch_d[:])
            nc.sync.dma_start(wall_s[:], wall_d[:])
            nc.sync.dma_start(gamma_s[:], gamma_d[:])
            nc.sync.dma_start(beta_s[:], beta_d[:])
            nc.sync.dma_start(bs_s[:], bs_d[:])
            nc.sync.dma_start(linw_s[:], linw_d[:])
            nc.sync.dma_start(linb_s[:], linb_d[:])
            # x -> H  (node-major tiles: H[p, t*F + f] = x[t*128+p, f])
            nc.sync.dma_start(
                H[:].rearrange("p (t f) -> p t f", f=F),
                x_sh[:].rearrange("(t p) f -> p t f", p=128),
            )
            make_identity(nc, ident[:])
            nc.gpsimd.memset(eps_s[:], EPS)
            nc.gpsimd.memset(ones1[:], 1.0)
            nc.gpsimd.memset(ones128[:], 1.0)
            # iota_w[p, j*WIN + k] = k  (repeating 0..WIN-1)
            nc.gpsimd.iota(
                iota_w[:].rearrange("p (a b) -> p a b", b=WIN),
                pattern=[[0, TJ], [1, WIN]],
                base=0,
                channel_multiplier=0,
                allow_small_or_imprecise_dtypes=True,
            )
            nc.gpsimd.iota(
                iota_g[:],
                pattern=[[1, NH * GH]],
                base=0,
                channel_multiplier=0,
                allow_small_or_imprecise_dtypes=True,
            )

            with (
                tc.tile_pool(name="gath", bufs=2) as gathp,
                tc.tile_pool(name="smat", bufs=3) as smatp,
                tc.tile_pool(name="work", bufs=4) as workp,
                tc.tile_pool(name="small", bufs=4) as smallp,
                tc.tile_pool(name="psA", bufs=2, space="PSUM") as psA,
                tc.tile_pool(name="psagg", bufs=2, space="PSUM") as psagg,
            ):
                for layer in range(N_LAYERS):
                    wl = wall_s[:, layer * F:(layer + 1) * F]
                    # -- phase A: H' = H @ W_l (per tile, via PE transposes) --
                    for t in range(NT) if not globals().get("SKIP_PHASEA") else []:
                        hcols = H[:, t * F:(t + 1) * F]
                        pt1 = psA.tile([128, 128], F32, tag="pt1")
                        nc.tensor.transpose(pt1[:], hcols, ident[:])
                        ht = workp.tile([128, 128], F32, tag="ht")
                        nc.scalar.copy(ht[:], pt1[:])
                        pm = psA.tile([128, 128], F32, tag="pm")
                        nc.tensor.matmul(
                            pm[:], lhsT=wl, rhs=ht[:], start=True, stop=True
                        )
                        hpt = workp.tile([128, 128], F32, tag="hpt")
                        nc.scalar.copy(hpt[:], pm[:])
                        pt2 = psA.tile([128, 128], F32, tag="pt2")
                        nc.tensor.transpose(pt2[:], hpt[:], ident[:])
                        hrow = workp.tile([128, 128], F32, tag="hrow")
                        nc.vector.tensor_copy(hrow[:], pt2[:])
                        nc.sync.dma_start(
                            hp_local[layer][t * 128:(t + 1) * 128, :], hrow[:]
                        )

                    # -- AllGather: every core gets the full H' table --
                    if globals().get("COLL_MODE") == "local":
                        nc.sync.dma_start(
                            hp_full[layer][0:LPAD, :], hp_local[layer][:]
                        )
                    else:
                        nc.gpsimd.collective_compute(
                            "AllGather",
                            mybir.AluOpType.bypass,
                            replica_groups=[list(range(NC))],
                            ins=[hp_local[layer][:].opt()],
                            outs=[hp_full[layer][:].opt()],
                        )

                    if debug and layer == 0:
                        nc.sync.dma_start(dbg_hp[:], hp_local[0][:])
                    # -- phase B: message aggregation + LN + residual --
                    for t in range(NT):
                        cols = slice(t * TJ, (t + 1) * TJ)
                        gmode = globals().get("GATHER_MODE", "indirect")
                        if gmode == "skip":
                            gat = gatfix
                        else:
                            gat = gathp.tile([128, TJ * 128], F32, tag="gat")
                        if gmode == "indirect":
                            for j in range(TJ):
                                nc.gpsimd.indirect_dma_start(
                                    out=gat[:, j * 128:(j + 1) * 128],
                                    out_offset=None,
                                    in_=hp_full[layer][:],
                                    in_offset=bass.IndirectOffsetOnAxis(
                                        ap=esrc_s[:, t * TJ + j:t * TJ + j + 1],
                                        axis=0,
                                    ),
                                )
                        elif gmode == "dense":
                            nc.sync.dma_start(
                                gat[:].rearrange("p (a b) -> p a b", b=128),
                                hp_full[layer][t * TJ * 128:(t + 1) * TJ * 128, :]
                                .rearrange("(a p) b -> p a b", p=128),
                            )
                        # gmode == "skip": no gather at all
                        # S[e, j, k] = (erel[e, j] == k) * norm[e, j]
                        smat = smatp.tile([128, TJ * WIN], F32, tag="smat")
                        smat3 = smat[:].rearrange("p (a b) -> p a b", b=WIN)
                        if globals().get("SKIP_SBUILD"):
                            nc.gpsimd.memset(smat[:, 0:WIN], 0.0)
                        _d = nc.vector.tensor_tensor if not globals().get("SKIP_SBUILD") else (lambda **kw: None)
                        _d(
                            out=smat3,
                            in0=iota_w[:].rearrange("p (a b) -> p a b", b=WIN),
                            in1=_bcast_inner(erel_s[:, cols], WIN),
                            op=mybir.AluOpType.is_equal,
                        )
                        _d(
                            out=smat3,
                            in0=smat3,
                            in1=_bcast_inner(enorm_s[:, cols], WIN),
                            op=mybir.AluOpType.mult,
                        )
                        agg = psagg.tile([128, 128], F32, tag="agg")
                        # bias (rank-1) also zero-initializes the full psum
                        nc.tensor.matmul(
                            agg[:], lhsT=ones1[:],
                            rhs=bs_s[:, layer * F:(layer + 1) * F],
                            start=True, stop=False, skip_group_check=True,
                        )
                        for w in range(NWIN) if not globals().get("SKIP_AGG") else []:
                            for k in range(CPW):
                                j = w * CPW + k
                                nc.tensor.matmul(
                                    agg[w * WIN:(w + 1) * WIN, :],
                                    lhsT=smat[:, 0:WIN] if globals().get("SKIP_SBUILD") else smat[:, j * WIN:(j + 1) * WIN],
                                    rhs=gat[:, j * 128:(j + 1) * 128],
                                    start=False, stop=j == TJ - 1,
                                    skip_group_check=True,
                                )
                        # ---- LayerNorm ----
                        rowsum = smallp.tile([128, 1], F32, tag="rowsum")
                        nc.vector.reduce_sum(
                            rowsum[:], agg[:], axis=mybir.AxisListType.X
                        )
                        mean = smallp.tile([128, 1], F32, tag="mean")
                        nc.scalar.activation(
                            mean[:], rowsum[:],
                            mybir.ActivationFunctionType.Copy, scale=1.0 / F,
                        )
                        xc = workp.tile([128, 128], F32, tag="xc")
                        nc.vector.tensor_scalar(
                            out=xc[:], in0=agg[:], scalar1=mean[:, 0:1],
                            scalar2=None, op0=mybir.AluOpType.subtract,
                        )
                        sq = workp.tile([128, 128], F32, tag="sq")
                        sqsum = smallp.tile([128, 1], F32, tag="sqsum")
                        nc.scalar.activation(
                            sq[:], xc[:], mybir.ActivationFunctionType.Square,
                            accum_out=sqsum[:],
                        )
                        std = smallp.tile([128, 1], F32, tag="std")
                        nc.scalar.activation(
                            std[:], sqsum[:], mybir.ActivationFunctionType.Sqrt,
                            bias=eps_s[:, 0:1], scale=1.0 / F,
                        )
                        rstd = smallp.tile([128, 1], F32, tag="rstd")
                        nc.vector.reciprocal(rstd[:], std[:])
                        hn = workp.tile([128, 128], F32, tag="hn")
                        nc.scalar.activation(
                            hn[:], xc[:], mybir.ActivationFunctionType.Copy,
                            scale=rstd[:, 0:1],
                        )
                        gl = gamma_s[:, layer * F:(layer + 1) * F]
                        bl = beta_s[:, layer * F:(layer + 1) * F]
                        gm = workp.tile([128, 128], F32, tag="gm")
                        nc.vector.tensor_tensor(
                            out=gm[:], in0=hn[:], in1=gl,
                            op=mybir.AluOpType.mult,
                        )
                        nc.vector.tensor_tensor(
                            out=gm[:], in0=gm[:], in1=bl,
                            op=mybir.AluOpType.add,
                        )
                        hcols = H[:, t * F:(t + 1) * F]
                        if layer == 0:
                            nc.scalar.activation(
                                hcols, gm[:],
                                mybir.ActivationFunctionType.Relu,
                            )
                        elif layer < N_LAYERS - 1:
                            rl = workp.tile([128, 128], F32, tag="rl")
                            nc.scalar.activation(
                                rl[:], gm[:],
                                mybir.ActivationFunctionType.Relu,
                            )
                            nc.vector.tensor_tensor(
                                out=hcols, in0=hcols, in1=rl[:],
                                op=mybir.AluOpType.add,
                            )
                        else:
                            nc.vector.tensor_tensor(
                                out=hcols, in0=hcols, in1=gm[:],
                                op=mybir.AluOpType.add,
                            )

                    if debug and layer == 0:
                        nc.sync.dma_start(
                            dbg_h1[:].rearrange("(t p) f -> p t f", p=128),
                            H[:].rearrange("p (t f) -> p t f", f=F),
                        )

            # ---- global mean pool + linear head ----
            last_tile = {}
            for t in range(NT):
                for h in pool_plan[t]:
                    last_tile[h] = t
            with (
                tc.tile_pool(name="sgp", bufs=3) as sgp,
                tc.tile_pool(name="pspool", bufs=1, space="PSUM") as pspool,
            ):
                psum_pool = [
                    pspool.tile([128, GH], F32, tag=f"pool{h}",
                                name=f"pool{h}")
                    for h in range(NH)
                ]
                psum_cnt = [
                    pspool.tile([1, GH], F32, tag=f"cnt{h}", name=f"cnt{h}")
                    for h in range(NH)
                ]
                started = [False] * NH
                for t in range(NT):
                    for h in pool_plan[t]:
                        sg = sgp.tile([128, GH], F32, tag="sg")
                        nc.vector.tensor_tensor(
                            out=sg[:],
                            in0=iota_g[:, h * GH:(h + 1) * GH],
                            in1=batch_s[:, t:t + 1].to_broadcast([128, GH]),
                            op=mybir.AluOpType.is_equal,
                        )
                        nc.tensor.matmul(
                            psum_pool[h][:], lhsT=H[:, t * F:(t + 1) * F],
                            rhs=sg[:], start=not started[h],
                            stop=last_tile[h] == t, skip_group_check=True,
                        )
                        nc.tensor.matmul(
                            psum_cnt[h][:], lhsT=ones128[:], rhs=sg[:],
                            start=not started[h], stop=last_tile[h] == t,
                            skip_group_check=True,
                        )
                        started[h] = True

                packed = constp.tile([128, NH * GH], F32, tag="packed")
                cnt_sb = constp.tile([1, NH * GH], F32, tag="cntsb")
                for h in range(NH):
                    nc.scalar.copy(
                        packed[0:128, h * GH:(h + 1) * GH], psum_pool[h][:]
                    )
                    nc.vector.tensor_copy(
                        cnt_sb[:, h * GH:(h + 1) * GH], psum_cnt[h][:]
                    )
                nc.sync.dma_start(ar_in[0:128, :], packed[:])
                nc.sync.dma_start(ar_in[128:129, :], cnt_sb[:])
                if globals().get("COLL_MODE") == "local":
                    nc.sync.dma_start(ar_out[:], ar_in[:])
                else:
                    nc.gpsimd.collective_compute(
                        "AllReduce",
                        mybir.AluOpType.add,
                        replica_groups=[list(range(NC))],
                        ins=[ar_in[:].opt()],
                        outs=[ar_out[:].opt()],
                    )
                red = constp.tile([128, NH * GH], F32, tag="red")
                redc = constp.tile([1, NH * GH], F32, tag="redc")
                nc.sync.dma_start(red[:], ar_out[0:128, :])
                nc.sync.dma_start(redc[:], ar_out[128:129, :])
                # dot with lin_w: [1, NH*GH]
                dots = constp.tile([1, NH * GH], F32, tag="dots")
                for h in range(NH):
                    pdot = pspool.tile([1, GH], F32, tag="pdot")
                    nc.tensor.matmul(
                        pdot[:], lhsT=linw_s[:],
                        rhs=red[0:128, h * GH:(h + 1) * GH],
                        start=True, stop=True, skip_group_check=True,
                    )
                    nc.scalar.copy(dots[:, h * GH:(h + 1) * GH], pdot[:])
                cnt = constp.tile([1, NH * GH], F32, tag="cntrow")
                nc.vector.tensor_scalar(
                    out=cnt[:], in0=redc[:], scalar1=1.0, scalar2=None,
                    op0=mybir.AluOpType.max,
                )
                cinv = constp.tile([1, NH * GH], F32, tag="cinv")
                nc.vector.reciprocal(cinv[:], cnt[:])
                res = constp.tile([1, NH * GH], F32, tag="res")
                nc.vector.tensor_tensor(
                    out=res[:], in0=dots[:], in1=cinv[:],
                    op=mybir.AluOpType.mult,
                )
                nc.vector.tensor_scalar(
                    out=res[:], in0=res[:], scalar1=linb_s[0:1, 0:1],
                    scalar2=None, op0=mybir.AluOpType.add,
                )
                nc.sync.dma_start(
                    out_d[:].rearrange("n one -> one n"),
                    res[:, 0:N_GRAPHS],
                )
    if not nc.is_finalized():
        nc.finalize()
    return nc


_CACHE = {}


def kernel(x, edge_index, batch, w1, ws, bs, gammas, betas, lin_w, lin_b):
    x = np.asarray(x, dtype=np.float32)
    edge_index = np.asarray(edge_index, dtype=np.int64)
    batch = np.asarray(batch, dtype=np.int64)
    per_core, CPW = _preprocess(edge_index, batch)
    pool_plan = _pool_plan(batch)

    wall = np.concatenate(
        [np.asarray(w1, np.float32)]
        + [np.asarray(ws[i], np.float32) for i in range(6)],
        axis=1,
    )  # [F, 7F]
    gamma_rep = np.concatenate(
        [np.tile(np.asarray(gammas[l], np.float32)[None, :], (128, 1))
         for l in range(7)], axis=1)
    beta_rep = np.concatenate(
        [np.tile(np.asarray(betas[l], np.float32)[None, :], (128, 1))
         for l in range(7)], axis=1)
    bsr = np.asarray(bs, np.float32)

    debug = bool(globals().get("DEBUG"))
    key = (CPW, tuple(tuple(p) for p in pool_plan), debug,
           globals().get("GATHER_MODE", "indirect"),
           globals().get("COLL_MODE", "cc"),
           bool(globals().get("SKIP_PHASEA")), bool(globals().get("SKIP_SBUILD")),
           bool(globals().get("SKIP_AGG")))
    if key not in _CACHE:
        _CACHE[key] = _build_nc(CPW, pool_plan, debug)
    nc = _CACHE[key]

    in_maps = []
    for c in range(NC):
        xp = np.zeros((LPAD, F), dtype=np.float32)
        xp[:LOCAL] = x[c * LOCAL:(c + 1) * LOCAL]
        bv = np.full((LPAD,), 2 * N_GRAPHS, dtype=np.float32)
        bv[:LOCAL] = batch[c * LOCAL:(c + 1) * LOCAL].astype(np.float32)
        batchv = np.ascontiguousarray(bv.reshape(NT, 128).T)  # [128, NT]
        in_maps.append({
            "x_sh": xp,
            "esrc": per_core[c]["esrc"],
            "enorm": per_core[c]["enorm"],
            "erel": per_core[c]["erel"],
            "batchv": batchv,
            "wall": wall,
            "gammar": gamma_rep,
            "betar": beta_rep,
            "bsr": bsr.reshape(1, N_LAYERS * F),
            "linw": np.asarray(lin_w, np.float32).reshape(F, 1),
            "linb": np.asarray(lin_b, np.float32).reshape(1, 1),
        })

    res = run_bass_kernel_spmd(
        nc, in_maps, core_ids=list(range(NC)), trace=bool(globals().get("TRACE"))
    )
    globals()["LAST_RESULT"] = res
    return np.asarray(res.results[0]["out"], dtype=np.float32)


# revision 24
# speedup vs baseline: 1.0352x; 1.0352x over previous
"""MathildeGCN Trainium2 kernel: 7-layer GCN + global mean pool + linear head.

Strategy (8 NeuronCores, SPMD):
  - Nodes sharded contiguously: 12500/core, padded to 12544 = 98 tiles of 128.
  - Edges partitioned by dst shard, sorted by dst, bucketed into 32-node
    windows so the segment-sum becomes static-offset PSUM matmuls with
    one-hot (norm-scaled) selection matrices built on the vector engine.
  - Per layer: local H @ W (via PE transposes), AllGather of H' into a full
    gather table in DRAM, indirect-DMA gather of edge source rows, selection
    matmuls accumulate messages per dst tile, bias via rank-1 matmul,
    LayerNorm + ReLU + residual on-chip.
  - Mean-pool partials per graph via one-hot matmuls, AllReduce, final dot.
"""

import sys

sys.path.insert(0, "/opt/trn_rl_repo")

import numpy as np

import concourse.bass as bass
import concourse.bacc as bacc
import concourse.mybir as mybir
import concourse.tile as tile
from concourse.bass_utils import run_bass_kernel_spmd
from concourse.masks import make_identity

F32 = mybir.dt.float32
I32 = mybir.dt.int32

N_NODES = 100000
N_EDGES = 1600000
F = 128
N_GRAPHS = 1000
EPS = 1e-5
NC = 8
LOCAL = N_NODES // NC          # 12500
NT = (LOCAL + 127) // 128      # 98 tiles per core
LPAD = NT * 128                # 12544
GPAD = LPAD * NC               # 100352
WIN = 64                       # dst window width (selection matrix cols)
NWIN = 128 // WIN              # windows per tile (4)
NBUK = NT * NWIN               # buckets per core (392)
N_LAYERS = 7
GH = 512                       # graph window for pooling matmuls
NH = (N_GRAPHS + GH - 1) // GH


def _preprocess(edge_index, batch):
    """Build per-core edge arrays. Returns (per_core dict list, CPW)."""
    src = np.concatenate([edge_index[0], np.arange(N_NODES, dtype=np.int64)])
    dst = np.concatenate([edge_index[1], np.arange(N_NODES, dtype=np.int64)])
    deg = np.bincount(dst, minlength=N_NODES).astype(np.float64)
    dinv = np.where(deg > 0, 1.0 / np.sqrt(deg), 0.0)
    norm = (dinv[src] * dinv[dst]).astype(np.float32)
    # padded global ids for the gather table
    src_gid = ((src // LOCAL) * LPAD + (src % LOCAL)).astype(np.int32)

    cores = []
    all_counts = []
    for c in range(NC):
        lo, hi = c * LOCAL, (c + 1) * LOCAL
        m = (dst >= lo) & (dst < hi)
        ed = (dst[m] - lo).astype(np.int64)
        es = src_gid[m]
        en = norm[m]
        order = np.argsort(ed, kind="stable")
        ed, es, en = ed[order], es[order], en[order]
        buk = ed // WIN                      # bucket id 0..NBUK-1, sorted
        rel = (ed % WIN).astype(np.float32)  # dst index within window
        starts = np.searchsorted(buk, np.arange(NBUK))
        ends = np.searchsorted(buk, np.arange(NBUK) + 1)
        counts = ends - starts
        all_counts.append(counts)
        cores.append(dict(es=es, en=en, rel=rel, starts=starts, counts=counts))

    maxcnt = max(int(cnt.max()) for cnt in all_counts)
    CPW = (maxcnt + 127) // 128  # chunks per window (uniform, SPMD-safe)

    per_core = []
    for c in range(NC):
        d = cores[c]
        cap = CPW * 128
        esP = np.zeros((NBUK, cap), dtype=np.int32)
        enP = np.zeros((NBUK, cap), dtype=np.float32)
        relP = np.zeros((NBUK, cap), dtype=np.float32)
        pos = np.arange(len(d["es"])) - np.repeat(d["starts"], d["counts"])
        bix = np.repeat(np.arange(NBUK), d["counts"])
        esP[bix, pos] = d["es"]
        enP[bix, pos] = d["en"]
        relP[bix, pos] = d["rel"]

        # SBUF layout [128 lanes, NBUK*CPW cols]: col j = bucket*CPW + chunk
        def lanes(a):
            return np.ascontiguousarray(a.reshape(NBUK * CPW, 128).T)

        per_core.append(dict(esrc=lanes(esP), enorm=lanes(enP), erel=lanes(relP)))
    return per_core, CPW


def _pool_plan(batch):
    """Union over cores of per-tile graph halves (width GH) the tile touches."""
    plan = [set() for _ in range(NT)]
    for c in range(NC):
        lo = c * LOCAL
        for t in range(NT):
            a = lo + t * 128
            b = min(a + 128, lo + LOCAL)
            if a >= lo + LOCAL:
                continue
            gmin = int(batch[a]) // GH
            gmax = int(batch[b - 1]) // GH
            for h in range(gmin, min(gmax, NH - 1) + 1):
                plan[t].add(h)
    return [sorted(s) for s in plan]


def _bcast_inner(ap, reps):
    """Append a 0-stride inner dim of size `reps` to an AP."""
    new_ap = [list(p) for p in ap.ap] + [[0, reps]]
    return bass.AP(ap.tensor, ap.offset, new_ap)


def _build_nc(CPW, pool_plan, debug=False):
    NJ = NBUK * CPW          # edge-metadata columns in SBUF
    TJ = NWIN * CPW          # chunks per tile
    nc = bacc.Bacc(None, target_bir_lowering=False, num_devices=NC)

    # ---- I/O ----
    x_sh = nc.dram_tensor("x_sh", [LPAD, F], F32, kind="ExternalInput")
    esrc_d = nc.dram_tensor("esrc", [128, NJ], I32, kind="ExternalInput")
    enorm_d = nc.dram_tensor("enorm", [128, NJ], F32, kind="ExternalInput")
    erel_d = nc.dram_tensor("erel", [128, NJ], F32, kind="ExternalInput")
    batch_d = nc.dram_tensor("batchv", [128, NT], F32, kind="ExternalInput")
    wall_d = nc.dram_tensor("wall", [F, N_LAYERS * F], F32, kind="ExternalInput")
    gamma_d = nc.dram_tensor("gammar", [128, N_LAYERS * F], F32, kind="ExternalInput")
    beta_d = nc.dram_tensor("betar", [128, N_LAYERS * F], F32, kind="ExternalInput")
    bs_d = nc.dram_tensor("bsr", [1, N_LAYERS * F], F32, kind="ExternalInput")
    linw_d = nc.dram_tensor("linw", [F, 1], F32, kind="ExternalInput")
    linb_d = nc.dram_tensor("linb", [1, 1], F32, kind="ExternalInput")
    out_d = nc.dram_tensor("out", [N_GRAPHS, 1], F32, kind="ExternalOutput")
    if debug:
        dbg_hp = nc.dram_tensor("dbg_hp", [LPAD, F], F32, kind="ExternalOutput")
        dbg_h1 = nc.dram_tensor("dbg_h1", [LPAD, F], F32, kind="ExternalOutput")

    # per-layer AllGather buffers (dedicated tensors: offset-0 APs for the
    # indirect gather, and no cross-layer WAR hazards)
    hp_local = [
        nc.dram_tensor(f"hploc{l}", [LPAD, F], F32) for l in range(N_LAYERS)
    ]
    hp_full = [
        nc.dram_tensor(f"hpfull{l}", [GPAD, F], F32, addr_space="Shared")
        for l in range(N_LAYERS)
    ]
    ar_in = nc.dram_tensor("ar_in", [129, NH * GH], F32)
    ar_out = nc.dram_tensor("ar_out", [129, NH * GH], F32, addr_space="Shared")

    with tile.TileContext(nc) as tc:
        with tc.tile_pool(name="const", bufs=1) as constp:
            # ---- persistent SBUF ----
            H = constp.tile([128, NT * F], F32, tag="H")
            esrc_s = constp.tile([128, NJ], I32, tag="esrc")
            enorm_s = constp.tile([128, NJ], F32, tag="enorm")
            erel_s = constp.tile([128, NJ], F32, tag="erel")
            batch_s = constp.tile([128, NT], F32, tag="batch")
            wall_s = constp.tile([F, N_LAYERS * F], F32, tag="wall")
            gamma_s = constp.tile([128, N_LAYERS * F], F32, tag="gamma")
            beta_s = constp.tile([128, N_LAYERS * F], F32, tag="beta")
            bs_s = constp.tile([1, N_LAYERS * F], F32, tag="bs")
            linw_s = constp.tile([F, 1], F32, tag="linw")
            linb_s = constp.tile([1, 1], F32, tag="linb")
            ident = constp.tile([128, 128], F32, tag="ident")
            ones1 = constp.tile([1, 128], F32, tag="ones1")
            ones128 = constp.tile([128, 1], F32, tag="ones128")
            iota_w = constp.tile([128, TJ * WIN], F32, tag="iotaw")
            iota_g = constp.tile([128, NH * GH], F32, tag="iotag")
            eps_s = constp.tile([128, 1], F32, tag="eps")
            if globals().get("GATHER_MODE") == "skip":
                gatfix = constp.tile([128, NWIN * CPW * 128], F32, tag="gatfix")
                nc.gpsimd.memset(gatfix[:], 1.0)

            nc.sync.dma_start(esrc_s[:], esrc_d[:])
            nc.sync.dma_start(enorm_s[:], enorm_d[:])
            nc.sync.dma_start(erel_s[:], erel_d[:])
            nc.sync.dma_start(batch_s[:], batch_d[:])
            nc.sync.dma_start(wall_s[:], wall_d[:])
            nc.sync.dma_start(gamma_s[:], gamma_d[:])
            nc.sync.dma_start(beta_s[:], beta_d[:])
            nc.sync.dma_start(bs_s[:], bs_d[:])
            nc.sync.dma_start(linw_s[:], linw_d[:])
            nc.sync.dma_start(linb_s[:], linb_d[:])
            # x -> H  (node-major tiles: H[p, t*F + f] = x[t*128+p, f])
            nc.sync.dma_start(
                H[:].rearrange("p (t f) -> p t f", f=F),
                x_sh[:].rearrange("(t p) f -> p t f", p=128),
            )
            make_identity(nc, ident[:])
            nc.gpsimd.memset(eps_s[:], EPS)
            nc.gpsimd.memset(ones1[:], 1.0)
            nc.gpsimd.memset(ones128[:], 1.0)
            # iota_w[p, j*WIN + k] = k  (repeating 0..WIN-1)
            nc.gpsimd.iota(
                iota_w[:].rearrange("p (a b) -> p a b", b=WIN),
                pattern=[[0, TJ], [1, WIN]],
                base=0,
                channel_multiplier=0,
                allow_small_or_imprecise_dtypes=True,
            )
            nc.gpsimd.iota(
                iota_g[:],
                pattern=[[1, NH * GH]],
                base=0,
                channel_multiplier=0,
                allow_small_or_imprecise_dtypes=True,
            )

            with (
                tc.tile_pool(name="gath", bufs=2) as gathp,
                tc.tile_pool(name="smat", bufs=3) as smatp,
                tc.tile_pool(name="work", bufs=4) as workp,
                tc.tile_pool(name="small", bufs=4) as smallp,
                tc.tile_pool(name="psA", bufs=2, space="PSUM") as psA,
                tc.tile_pool(name="psagg", bufs=2, space="PSUM") as psagg,
            ):
                for layer in range(N_LAYERS):
                    wl = wall_s[:, layer * F:(layer + 1) * F]
                    # -- phase A: H' = H @ W_l (per tile, via PE transposes) --
                    for t in range(NT) if not globals().get("SKIP_PHASEA") else []:
                        hcols = H[:, t * F:(t + 1) * F]
                        pt1 = psA.tile([128, 128], F32, tag="pt1")
                        nc.tensor.transpose(pt1[:], hcols, ident[:])
                        ht = workp.tile([128, 128], F32, tag="ht")
                        nc.scalar.copy(ht[:], pt1[:])
                        pm = psA.tile([128, 128], F32, tag="pm")
                        nc.tensor.matmul(
                            pm[:], lhsT=wl, rhs=ht[:], start=True, stop=True
                        )
                        hpt = workp.tile([128, 128], F32, tag="hpt")
                        nc.scalar.copy(hpt[:], pm[:])
                        pt2 = psA.tile([128, 128], F32, tag="pt2")
                        nc.tensor.transpose(pt2[:], hpt[:], ident[:])
                        hrow = workp.tile([128, 128], F32, tag="hrow")
                        nc.vector.tensor_copy(hrow[:], pt2[:])
                        nc.sync.dma_start(
                            hp_local[layer][t * 128:(t + 1) * 128, :], hrow[:]
                        )

                    # -- AllGather: every core gets the full H' table --
                    if globals().get("COLL_MODE") == "local":
                        nc.sync.dma_start(
                            hp_full[layer][0:LPAD, :], hp_local[layer][:]
                        )
                    else:
                        nc.gpsimd.collective_compute(
                            "AllGather",
                            mybir.AluOpType.bypass,
                            replica_groups=[list(range(NC))],
                            ins=[hp_local[layer][:].opt()],
                            outs=[hp_full[layer][:].opt()],
                        )

                    if debug and layer == 0:
                        nc.sync.dma_start(dbg_hp[:], hp_local[0][:])
                    # -- phase B: message aggregation + LN + residual --
                    for t in range(NT):
                        cols = slice(t * TJ, (t + 1) * TJ)
                        gmode = globals().get("GATHER_MODE", "indirect")
                        if gmode == "skip":
                            gat = gatfix
                        else:
                            gat = gathp.tile([128, TJ * 128], F32, tag="gat")
                        if gmode == "indirect":
                            for j in range(TJ):
                                nc.gpsimd.indirect_dma_start(
                                    out=gat[:, j * 128:(j + 1) * 128],
                                    out_offset=None,
                                    in_=hp_full[layer][:],
                                    in_offset=bass.IndirectOffsetOnAxis(
                                        ap=esrc_s[:, t * TJ + j:t * TJ + j + 1],
                                        axis=0,
                                    ),
                                )
                        elif gmode == "dense":
                            nc.sync.dma_start(
                                gat[:].rearrange("p (a b) -> p a b", b=128),
                                hp_full[layer][t * TJ * 128:(t + 1) * TJ * 128, :]
                                .rearrange("(a p) b -> p a b", p=128),
                            )
                        # gmode == "skip": no gather at all
                        # S[e, j, k] = (erel[e, j] == k) * norm[e, j]
                        smat = smatp.tile([128, TJ * WIN], F32, tag="smat")
                        smat3 = smat[:].rearrange("p (a b) -> p a b", b=WIN)
                        if globals().get("SKIP_SBUILD"):
                            nc.gpsimd.memset(smat[:, 0:WIN], 0.0)
                        _d = nc.vector.tensor_tensor if not globals().get("SKIP_SBUILD") else (lambda **kw: None)
                        _d(
                            out=smat3,
                            in0=iota_w[:].rearrange("p (a b) -> p a b", b=WIN),
                            in1=_bcast_inner(erel_s[:, cols], WIN),
                            op=mybir.AluOpType.is_equal,
                        )
                        _d(
                            out=smat3,
                            in0=smat3,
                            in1=_bcast_inner(enorm_s[:, cols], WIN),
                            op=mybir.AluOpType.mult,
                        )
                        agg = psagg.tile([128, 128], F32, tag="agg")
                        # bias (rank-1) also zero-initializes the full psum
                        nc.tensor.matmul(
                            agg[:], lhsT=ones1[:],
                            rhs=bs_s[:, layer * F:(layer + 1) * F],
                            start=True, stop=False, skip_group_check=True,
                        )
                        for w in range(NWIN) if not globals().get("SKIP_AGG") else []:
                            for k in range(CPW):
                                j = w * CPW + k
                                nc.tensor.matmul(
                                    agg[w * WIN:(w + 1) * WIN, :],
                                    lhsT=smat[:, 0:WIN] if globals().get("SKIP_SBUILD") else smat[:, j * WIN:(j + 1) * WIN],
                                    rhs=gat[:, j * 128:(j + 1) * 128],
                                    start=False, stop=j == TJ - 1,
                                    skip_group_check=True,
                                )
                        # ---- LayerNorm ----
                        rowsum = smallp.tile([128, 1], F32, tag="rowsum")
                        nc.vector.reduce_sum(
                            rowsum[:], agg[:], axis=mybir.AxisListType.X
                        )
                        mean = smallp.tile([128, 1], F32, tag="mean")
                        nc.scalar.activation(
                            mean[:], rowsum[:],
                            mybir.ActivationFunctionType.Copy, scale=1.0 / F,
                        )
                        xc = workp.tile([128, 128], F32, tag="xc")
                        nc.vector.tensor_scalar(
                            out=xc[:], in0=agg[:], scalar1=mean[:, 0:1],
                            scalar2=None, op0=mybir.AluOpType.subtract,
                        )
                        sq = workp.tile([128, 128], F32, tag="sq")
                        sqsum = smallp.tile([128, 1], F32, tag="sqsum")
                        nc.scalar.activation(
                            sq[:], xc[:], mybir.ActivationFunctionType.Square,
                            accum_out=sqsum[:],
                        )
                        std = smallp.tile([128, 1], F32, tag="std")
                        nc.scalar.activation(
                            std[:], sqsum[:], mybir.ActivationFunctionType.Sqrt,
                            bias=eps_s[:, 0:1], scale=1.0 / F,
                        )
                        rstd = smallp.tile([128, 1], F32, tag="rstd")
                        nc.vector.reciprocal(rstd[:], std[:])
                        hn = workp.tile([128, 128], F32, tag="hn")
                        nc.scalar.activation(
                            hn[:], xc[:], mybir.ActivationFunctionType.Copy,
                            scale=rstd[:, 0:1],
                        )
                        gl = gamma_s[:, layer * F:(layer + 1) * F]
                        bl = beta_s[:, layer * F:(layer + 1) * F]
                        gm = workp.tile([128, 128], F32, tag="gm")
                        nc.vector.tensor_tensor(
                            out=gm[:], in0=hn[:], in1=gl,
                            op=mybir.AluOpType.mult,
                        )
                        nc.vector.tensor_tensor(
                            out=gm[:], in0=gm[:], in1=bl,
                            op=mybir.AluOpType.add,
                        )
                        hcols = H[:, t * F:(t + 1) * F]
                        if layer == 0:
                            nc.scalar.activation(
                                hcols, gm[:],
                                mybir.ActivationFunctionType.Relu,
                            )
                        elif layer < N_LAYERS - 1:
                            rl = workp.tile([128, 128], F32, tag="rl")
                            nc.scalar.activation(
                                rl[:], gm[:],
                                mybir.ActivationFunctionType.Relu,
                            )
                            nc.vector.tensor_tensor(
                                out=hcols, in0=hcols, in1=rl[:],
                                op=mybir.AluOpType.add,
                            )
                        else:
                            nc.vector.tensor_tensor(
                                out=hcols, in0=hcols, in1=gm[:],
                                op=mybir.AluOpType.add,
                            )

                    if debug and layer == 0:
                        nc.sync.dma_start(
                            dbg_h1[:].rearrange("(t p) f -> p t f", p=128),
                            H[:].rearrange("p (t f) -> p t f", f=F),
                        )

            # ---- global mean pool + linear head ----
            last_tile = {}
            for t in range(NT):
                for h in pool_plan[t]:
                    last_tile[h] = t
            with (
                tc.tile_pool(name="sgp", bufs=3) as sgp,
                tc.tile_pool(name="pspool", bufs=1, space="PSUM") as pspool,
            ):
                psum_pool = [
                    pspool.tile([128, GH], F32, tag=f"pool{h}",
                                name=f"pool{h}")
                    for h in range(NH)
                ]
                psum_cnt = [
                    pspool.tile([1, GH], F32, tag=f"cnt{h}", name=f"cnt{h}")
                    for h in range(NH)
                ]
                started = [False] * NH
                for t in range(NT):
                    for h in pool_plan[t]:
                        sg = sgp.tile([128, GH], F32, tag="sg")
                        nc.vector.tensor_tensor(
                            out=sg[:],
                            in0=iota_g[:, h * GH:(h + 1) * GH],
                            in1=batch_s[:, t:t + 1].to_broadcast([128, GH]),
                            op=mybir.AluOpType.is_equal,
                        )
                        nc.tensor.matmul(
                            psum_pool[h][:], lhsT=H[:, t * F:(t + 1) * F],
                            rhs=sg[:], start=not started[h],
                            stop=last_tile[h] == t, skip_group_check=True,
                        )
                        nc.tensor.matmul(
                            psum_cnt[h][:], lhsT=ones128[:], rhs=sg[:],
                            start=not started[h], stop=last_tile[h] == t,
                            skip_group_check=True,
                        )
                        started[h] = True

                packed = constp.tile([128, NH * GH], F32, tag="packed")
                cnt_sb = constp.tile([1, NH * GH], F32, tag="cntsb")
                for h in range(NH):
                    nc.scalar.copy(
                        packed[0:128, h * GH:(h + 1) * GH], psum_pool[h][:]
                    )
                    nc.vector.tensor_copy(
                        cnt_sb[:, h * GH:(h + 1) * GH], psum_cnt[h][:]
                    )
                nc.sync.dma_start(ar_in[0:128, :], packed[:])
                nc.sync.dma_start(ar_in[128:129, :], cnt_sb[:])
                if globals().get("COLL_MODE") == "local":
                    nc.sync.dma_start(ar_out[:], ar_in[:])
                else:
                    nc.gpsimd.collective_compute(
                        "AllReduce",
                        mybir.AluOpType.add,
                        replica_groups=[list(range(NC))],
                        ins=[ar_in[:].opt()],
                        outs=[ar_out[:].opt()],
                    )
                red = constp.tile([128, NH * GH], F32, tag="red")
                redc = constp.tile([1, NH * GH], F32, tag="redc")
                nc.sync.dma_start(red[:], ar_out[0:128, :])
                nc.sync.dma_start(redc[:], ar_out[128:129, :])
                # dot with lin_w: [1, NH*GH]
                dots = constp.tile([1, NH * GH], F32, tag="dots")
                for h in range(NH):
                    pdot = pspool.tile([1, GH], F32, tag="pdot")
                    nc.tensor.matmul(
                        pdot[:], lhsT=linw_s[:],
                        rhs=red[0:128, h * GH:(h + 1) * GH],
                        start=True, stop=True, skip_group_check=True,
                    )
                    nc.scalar.copy(dots[:, h * GH:(h + 1) * GH], pdot[:])
                cnt = constp.tile([1, NH * GH], F32, tag="cntrow")
                nc.vector.tensor_scalar(
                    out=cnt[:], in0=redc[:], scalar1=1.0, scalar2=None,
                    op0=mybir.AluOpType.max,
                )
                cinv = constp.tile([1, NH * GH], F32, tag="cinv")
                nc.vector.reciprocal(cinv[:], cnt[:])
                res = constp.tile([1, NH * GH], F32, tag="res")
                nc.vector.tensor_tensor(
                    out=res[:], in0=dots[:], in1=cinv[:],
                    op=mybir.AluOpType.mult,
                )
                nc.vector.tensor_scalar(
                    out=res[:], in0=res[:], scalar1=linb_s[0:1, 0:1],
                    scalar2=None, op0=mybir.AluOpType.add,
                )
                nc.sync.dma_start(
                    out_d[:].rearrange("n one -> one n"),
                    res[:, 0:N_GRAPHS],
                )
    if not nc.is_finalized():
        nc.finalize()
    return nc


_CACHE = {}


def kernel(x, edge_index, batch, w1, ws, bs, gammas, betas, lin_w, lin_b):
    x = np.asarray(x, dtype=np.float32)
    edge_index = np.asarray(edge_index, dtype=np.int64)
    batch = np.asarray(batch, dtype=np.int64)
    per_core, CPW = _preprocess(edge_index, batch)
    pool_plan = _pool_plan(batch)

    wall = np.concatenate(
        [np.asarray(w1, np.float32)]
        + [np.asarray(ws[i], np.float32) for i in range(6)],
        axis=1,
    )  # [F, 7F]
    gamma_rep = np.concatenate(
        [np.tile(np.asarray(gammas[l], np.float32)[None, :], (128, 1))
         for l in range(7)], axis=1)
    beta_rep = np.concatenate(
        [np.tile(np.asarray(betas[l], np.float32)[None, :], (128, 1))
         for l in range(7)], axis=1)
    bsr = np.asarray(bs, np.float32)

    debug = bool(globals().get("DEBUG"))
    key = (CPW, tuple(tuple(p) for p in pool_plan), debug,
           globals().get("GATHER_MODE", "indirect"),
           globals().get("COLL_MODE", "cc"),
           bool(globals().get("SKIP_PHASEA")), bool(globals().get("SKIP_SBUILD")),
           bool(globals().get("SKIP_AGG")))
    if key not in _CACHE:
        _CACHE[key] = _build_nc(CPW, pool_plan, debug)
    nc = _CACHE[key]

    in_maps = []
    for c in range(NC):
        xp = np.zeros((LPAD, F), dtype=np.float32)
        xp[:LOCAL] = x[c * LOCAL:(c + 1) * LOCAL]
        bv = np.full((LPAD,), 2 * N_GRAPHS, dtype=np.float32)
        bv[:LOCAL] = batch[c * LOCAL:(c + 1) * LOCAL].astype(np.float32)
        batchv = np.ascontiguousarray(bv.reshape(NT, 128).T)  # [128, NT]
        in_maps.append({
            "x_sh": xp,
            "esrc": per_core[c]["esrc"],
            "enorm": per_core[c]["enorm"],
            "erel": per_core[c]["erel"],
            "batchv": batchv,
            "wall": wall,
            "gammar": gamma_rep,
            "betar": beta_rep,
            "bsr": bsr.reshape(1, N_LAYERS * F),
            "linw": np.asarray(lin_w, np.float32).reshape(F, 1),
            "linb": np.asarray(lin_b, np.float32).reshape(1, 1),
        })

    res = run_bass_kernel_spmd(
        nc, in_maps, core_ids=list(range(NC)), trace=bool(globals().get("TRACE"))
    )
    globals()["LAST_RESULT"] = res
    return np.asarray(res.results[0]["out"], dtype=np.float32)
